# revision 29
# baseline (speedup 1.0000x reference)
"""Trainium2 Bass kernel for nn_CrossAttentionModule_bias.

Math (B=2, C=256, H=W=64, N=4096):
    q = queries.reshape(B,C,N).T + q_pos        # [B,N,C]
    k = keys.reshape(B,C,N).T + k_pos
    v = values.reshape(B,C,N).T
    attn = softmax(q @ k.T / sqrt(C)) + c_b     # c_b: per-batch SCALAR
    out  = attn @ v   -> [B,C,H,W]

where c_b = softplus(bias_eye*s_eye) + softplus(bias_mouth*s_mouth), s_x =
sum(m*m) over the nearest-resized mask.  Adding the scalar c_b to every attn
entry adds c_b*S[c] (S = colsum V) to every output row; folded host-side into
V: v'[m,c] = v[m,c] + c_b*S[c] makes u'/Z = u/Z + c_b*S[c] exact.

Device kernel (per core, 8 cores = 2 batches x 4 query-column shards):
    dotsT[m,n] = sum_c keff[c,m] * qeff[c,n]    (bf16 matmuls, fp32 PSUM)
    e = exp(dotsT * 1/16) -> bf16               (no max subtraction, |dots|<18)
    U_T[c,n] accumulated over m-chunks on PE
    zsum[p,n] += e[p,n] on DVE (two independent chains to hide RMW latency)
    Zbc = ones128 @ (zsumA + zsumB) -> [128,n] broadcast of Z in one matmul
    out[c,n] = U_T[c,n] * recip(Zbc)[c,n]       (c_b*S folded into vaug)

Timing-critical structure: the QK pipeline runs `lookahead` steps ahead of
AV so PE never waits on ACT's exp latency; tails are deferred four steps so
PE doesn't stall on the DVE reciprocal; the For_i body holds `unroll` reps
to amortize the loop's all-engine barrier + pipeline fill/drain.

Measured attribution (HW slope, this box): pure matmul pipeline 55.7us
(= 256 MM x 218ns roofline; LDWEIGHTS free via FWL), +exp chain ~+13us
(ACT is a co-roofline: ~740ns per 512-wide op standalone, ~1075ns
effective in-kernel), +zsum ~+3us.  Rejected by A/B on HW: fp8/DoubleRow
matmuls (e4m3 QK or V alone -> 5.1e-2 rel err vs the 2e-2 gate; e5m2 expt
2.7e-2; hi/lo fp8 splits cost 1.13x bf16), 1024-wide exp super-tiles
(better standalone, worse in-kernel), zsum on GPSIMD, Z via GPSIMD
partition_all_reduce, deeper lookahead w/ shared zbc bank.
"""

import numpy as np
import ml_dtypes

import concourse.bass as bass
import concourse.mybir as mybir
import concourse.tile as tile
from concourse import bacc
from concourse import bass_isa
from concourse.bass_utils import run_bass_kernel_spmd

# Problem shape (hardcoded per the task contract)
B, C, H, W = 2, 256, 64, 64
N = H * W                      # 4096
NCORES = 8
SHARDS_PER_B = NCORES // B     # 4 query-column shards per batch
NSH = N // SHARDS_PER_B        # 1024 query columns per core
SCALE = float(C) ** -0.5       # 1/16
P = 128
CCN = C // P                   # 2 c-chunks
MCN = N // P                   # 32 m-chunks
NT_SIZE = 512                  # n-tile width (PSUM bank width in fp32)
NTN = NSH // NT_SIZE           # 2 n-tiles per core

F32 = mybir.dt.float32
F32R = mybir.dt.float32r
BF16 = mybir.dt.bfloat16

EXP = mybir.ActivationFunctionType.Exp

_CACHE: dict = {}


def _build_bass(reps: int = 1, loop_reps: int = 0, ablate: tuple = (),
                unroll: int = 16, lookahead: int = 2, dots_bufs: int = 3,
                z_split: bool = False, bc_share: bool = False,
                wpool_bufs: int = 6, tail_defer: int = 4,
                wide: bool = False, wtail_defer: int = 1, wexp_bufs: int = 4,
                z_allred: bool = False, av_lag: int = 1):
    """reps>1 unrolls the whole compute; loop_reps>0 wraps it in a hardware
    For_i loop (timing-only variants: slope between two loop_reps builds
    isolates per-iteration HW time from the ~100ms dispatch floor).  The
    For_i body holds `unroll` reps per iteration.

    z_split routes zsum chain 1 to GPSIMD (Pool) so DVE only carries one
    chain + the tails; dots_bufs=4 with zbc sharing the dots pool lets the
    QK pipeline run `lookahead`=3 deep (PSUM: 4 dots/zbc + 4 u = 8 banks)."""
    nc = bacc.Bacc("TRN2", target_bir_lowering=False, debug=False)

    keff = nc.dram_tensor("keff", [C, N], BF16, kind="ExternalInput")
    qeff = nc.dram_tensor("qeff", [C, NSH], BF16, kind="ExternalInput")
    vaug = nc.dram_tensor("vaug", [N, C], BF16, kind="ExternalInput")
    out = nc.dram_tensor("out", [C, NSH], F32, kind="ExternalOutput")

    KQ = 8                     # m-chunks per keff DMA tile
    KQN = MCN // KQ            # 4 keff tiles per c-chunk

    with tile.TileContext(nc) as tc:
        with (
            tc.tile_pool(name="const", bufs=1) as cpool,
            tc.tile_pool(name="work", bufs=wpool_bufs) as wpool,
            tc.tile_pool(name="zs", bufs=2) as zpool,
            tc.tile_pool(name="tail", bufs=3) as tpool,
            tc.tile_pool(name="dots_ps", bufs=(2 if wide else dots_bufs), space="PSUM") as dots_pool,
            tc.tile_pool(name="acc_ps", bufs=1, space="PSUM") as acc_pool,
            tc.tile_pool(name="bc_ps", bufs=1, space="PSUM") as bc_pool_sep,
        ):
            bc_pool, bc_tag = (dots_pool, "dots") if bc_share else (bc_pool_sep, "zbc")
            zero = cpool.tile([P, 1], F32, tag="zero", name="zero")
            nc.vector.memset(zero[:], 0.0)
            # all-ones [128,128] stationary: one matmul turns colsum+broadcast
            # of zsum into Zbc[j,n] = Z[n] on every partition j.  f32r matmul
            # operands must be produced as f32r; memset can't, so stage via copy.
            ones_f = cpool.tile([P, P], F32, tag="ones_f", name="ones_f")
            nc.vector.memset(ones_f[:], 1.0)
            ones128 = cpool.tile([P, P], F32R, tag="ones128", name="ones128")
            with nc.allow_low_precision(reason="f32r ones are exact"):
                nc.vector.tensor_copy(ones128[:], ones_f[:])

            # DMA issue order = first-use order, so the single-shot fill
            # (first LDWEIGHTS) doesn't wait behind bytes needed much later:
            # keff q=0 + qeff (first QK), first vaug chunks (first AVs),
            # then the remaining keff tiles, then the remaining vaug.
            KGROUPS = [(0, 8), (8, 16), (16, 24), (24, 32)]
            qeff_t = [None] * CCN
            keff_g = {}
            vaug_t = [None] * MCN

            def dma_qeff(cc):
                t = cpool.tile([P, NSH], BF16, tag=f"qeff{cc}", name=f"qeff{cc}")
                nc.sync.dma_start(t[:], qeff[cc * P : (cc + 1) * P, :])
                qeff_t[cc] = t

            def dma_keff(cc, gi):
                a, b = KGROUPS[gi]
                t = cpool.tile([P, (b - a) * P], BF16, tag=f"keff{cc}_{gi}",
                               name=f"keff{cc}_{gi}")
                nc.sync.dma_start(t[:], keff[cc * P : (cc + 1) * P, a * P : b * P])
                keff_g[(cc, gi)] = t

            def keff_slice(cc, mc):
                for gi, (a, b) in enumerate(KGROUPS):
                    if a <= mc < b:
                        return keff_g[(cc, gi)][:, (mc - a) * P : (mc - a + 1) * P]
                raise AssertionError(mc)

            def dma_vaug(mc):
                t = cpool.tile([P, C], BF16, tag=f"vaug{mc}", name=f"vaug{mc}")
                nc.sync.dma_start(t[:], vaug[mc * P : (mc + 1) * P, :])
                vaug_t[mc] = t

            for cc in range(CCN):
                dma_keff(cc, 0)
                dma_qeff(cc)
            for mc in range(4):
                dma_vaug(mc)
            for gi in range(1, len(KGROUPS)):
                for cc in range(CCN):
                    dma_keff(cc, gi)
            for mc in range(4, MCN):
                dma_vaug(mc)

            const_expt = None
            if "exp" in ablate:
                const_expt = cpool.tile([P, NT_SIZE], BF16, tag="cexpt", name="cexpt")
                nc.vector.memset(const_expt[:], 1.0)
            cexpsrc = None
            if "expdep" in ablate:
                # timing ablation: exp reads a constant SBUF tile instead of
                # the dots PSUM tile (keeps ACT work, breaks the dependency)
                cw = 2 * NT_SIZE if wide else NT_SIZE
                cexpsrc = cpool.tile([P, cw], F32, tag="cexpsrc", name="cexpsrc")
                nc.vector.memset(cexpsrc[:], 0.0)

            def emit_qk(nt, mc):
                ns = slice(nt * NT_SIZE, (nt + 1) * NT_SIZE)
                dots = dots_pool.tile([P, NT_SIZE], F32, tag="dots", name="dots")
                for cc in range(CCN):
                    lhsT = keff_slice(cc, mc)
                    nc.tensor.matmul(
                        dots[:],
                        lhsT,
                        qeff_t[cc][:, ns],
                        start=(cc == 0),
                        stop=(cc == CCN - 1),
                    )
                return dots

            def emit_body():
                u_ps = [
                    [
                        acc_pool.tile([P, NT_SIZE], F32, tag=f"u{cc}n{nt}", name=f"u{cc}n{nt}")
                        for cc in range(CCN)
                    ]
                    for nt in range(NTN)
                ]
                zsum = [[None, None] for _ in range(NTN)]  # two chains per nt

                def emit_tail(nt):
                    # Z colsum + partition-broadcast: ones-matmul pair into a
                    # PSUM bank, or (z_allred) DVE-combine + GPSIMD all-reduce
                    # which needs no PSUM bank and no PE work.
                    ns = slice(nt * NT_SIZE, (nt + 1) * NT_SIZE)
                    if z_allred:
                        zz = tpool.tile([P, NT_SIZE], F32, tag="zz", name="zz")
                        zbc = tpool.tile([P, NT_SIZE], F32, tag="zbcsb", name="zbcsb")
                        with nc.allow_low_precision(reason="z chains are f32r-tagged"):
                            nc.vector.tensor_add(zz[:], zsum[nt][0][:], zsum[nt][1][:])
                        nc.gpsimd.partition_all_reduce(
                            zbc[:], zz[:], P, bass_isa.ReduceOp.add
                        )
                    else:
                        zbc = bc_pool.tile([P, NT_SIZE], F32, tag=bc_tag, name="zbc")
                        nc.tensor.matmul(zbc[:], ones128[:], zsum[nt][0][:], start=True, stop=False)
                        nc.tensor.matmul(zbc[:], ones128[:], zsum[nt][1][:], start=False, stop=True)
                    recip = tpool.tile([P, NT_SIZE], F32, tag="recip", name="recip")
                    nc.vector.reciprocal(recip[:], zbc[:])
                    for cc in range(CCN):
                        outsb = tpool.tile([P, NT_SIZE], F32, tag="outsb", name="outsb")
                        nc.vector.tensor_mul(outsb[:], u_ps[nt][cc][:], recip[:])
                        nc.sync.dma_start(out[cc * P : (cc + 1) * P, ns], outsb[:])

                # software-pipelined `lookahead` deep; AV (and its zsum)
                # optionally lags `av_lag` steps behind exp so the ACT
                # latency has extra PE work to hide under; tails deferred
                # `tail_defer` AV-steps.
                steps = [(nt, mc) for nt in range(NTN) for mc in range(MCN)]
                pending_tail = []
                expt_q: dict = {}

                def emit_av_z(j):
                    nt, mc = steps[j]
                    expt = expt_q.pop(j)
                    first, last = mc == 0, mc == MCN - 1
                    for cc in range(CCN):
                        nc.tensor.matmul(
                            u_ps[nt][cc][:],
                            vaug_t[mc][:, cc * P : (cc + 1) * P],
                            expt[:],
                            start=first,
                            stop=last,
                        )
                    if "z" not in ablate:
                        ch = mc % 2
                        # chain 0 on DVE, chain 1 on GPSIMD (Pool) so neither
                        # engine carries the whole 64-op accumulation stream
                        zeng = nc.gpsimd if (z_split and ch == 1) else nc.vector
                        # f32r tag so the colsum matmul reads "rounded" input
                        with nc.allow_low_precision(reason="zsum feeds f32r matmul"):
                            if mc < 2:
                                zsum[nt][ch] = zpool.tile(
                                    [P, NT_SIZE], F32R, tag=f"zsum{ch}", name=f"zsum{ch}"
                                )
                                zeng.tensor_copy(zsum[nt][ch][:], expt[:])
                            else:
                                zeng.tensor_add(zsum[nt][ch][:], zsum[nt][ch][:], expt[:])
                    if pending_tail and pending_tail[0][0] <= j:
                        emit_tail(pending_tail.pop(0)[1])
                    if last and "tail" not in ablate and "z" not in ablate:
                        pending_tail.append((j + tail_defer, nt))

                dots_q = [emit_qk(*steps[j]) for j in range(lookahead)]
                for i in range(len(steps) + av_lag):
                    if i < len(steps):
                        dots = dots_q.pop(0)
                        if "exp" in ablate:
                            expt_q[i] = const_expt
                        else:
                            expt = wpool.tile([P, NT_SIZE], BF16, tag="expt", name="expt")
                            src_ap = cexpsrc if "expdep" in ablate else dots
                            nc.scalar.activation(
                                expt[:], src_ap[:], EXP, bias=zero[:], scale=SCALE
                            )
                            expt_q[i] = expt
                        if i + lookahead < len(steps):
                            dots_q.append(emit_qk(*steps[i + lookahead]))
                    j = i - av_lag
                    if 0 <= j < len(steps):
                        emit_av_z(j)
                for due, nt in pending_tail:
                    emit_tail(nt)

            def emit_body_wide():
                """Super-step body: two m-chunks per step share one
                [128,1024] dots PSUM super-tile (2 banks, QK matmuls write
                bank-aligned 512-halves) so exp is ONE 1024-wide ACT op and
                zsum ONE 1024-wide DVE op per super — halving the per-op
                fixed overhead (352 ACT cycles) that made ACT a co-roofline.
                AV lags 2 supers behind QK so exp latency is fully hidden
                with only 2 dots super-slots (4 banks; u takes the other 4).
                zsum2[nt] holds both parity chains side by side; Zbc sums
                the halves with two accumulating matmuls."""
                NT2 = 2 * NT_SIZE
                u_ps = [
                    [
                        acc_pool.tile([P, NT_SIZE], F32, tag=f"u{cc}n{nt}", name=f"u{cc}n{nt}")
                        for cc in range(CCN)
                    ]
                    for nt in range(NTN)
                ]
                zsum2 = [[None, None] for _ in range(NTN)]  # 2 chains (expt2 halves)

                def emit_qk_super(nt, ms):
                    ns = slice(nt * NT_SIZE, (nt + 1) * NT_SIZE)
                    d2 = dots_pool.tile([P, NT2], F32, tag="dots", name="dots2")
                    for h in range(2):
                        mc = 2 * ms + h
                        lo = h * NT_SIZE
                        for cc in range(CCN):
                            lhsT = keff_slice(cc, mc)
                            nc.tensor.matmul(
                                d2[:, lo : lo + NT_SIZE],
                                lhsT,
                                qeff_t[cc][:, ns],
                                start=(cc == 0),
                                stop=(cc == CCN - 1),
                            )
                    return d2

                def emit_av(i):
                    nt, ms = supers[i]
                    expt2 = expt_q[i]
                    for h in range(2):
                        mc = 2 * ms + h
                        lo = h * NT_SIZE
                        for cc in range(CCN):
                            nc.tensor.matmul(
                                u_ps[nt][cc][:],
                                vaug_t[mc][:, cc * P : (cc + 1) * P],
                                expt2[:, lo : lo + NT_SIZE],
                                start=(ms == 0 and h == 0),
                                stop=(ms == MCN // 2 - 1 and h == 1),
                            )
                    if "z" not in ablate:
                        # two independent 512-wide chains (one per expt2 half)
                        # keep the DVE RMW latency off the tail's critical path
                        for ch in range(2):
                            half = expt2[:, ch * NT_SIZE : (ch + 1) * NT_SIZE]
                            if ms == 0:
                                zsum2[nt][ch] = zpool.tile(
                                    [P, NT_SIZE], F32, tag=f"zsum2{ch}", name=f"zsum2{ch}"
                                )
                                nc.vector.tensor_copy(zsum2[nt][ch][:], half)
                            else:
                                nc.vector.tensor_add(zsum2[nt][ch][:], zsum2[nt][ch][:], half)

                def emit_tail(nt):
                    # Z: combine the parity chains on DVE, then GPSIMD
                    # all-reduce across partitions (replaces the ones-matmul
                    # Zbc: no PSUM bank, no PE work; ~3.3us hidden by defer)
                    ns = slice(nt * NT_SIZE, (nt + 1) * NT_SIZE)
                    zz = tpool.tile([P, NT_SIZE], F32, tag="zz", name="zz")
                    nc.vector.tensor_add(zz[:], zsum2[nt][0][:], zsum2[nt][1][:])
                    zbc = tpool.tile([P, NT_SIZE], F32, tag="zbcsb", name="zbcsb")
                    nc.gpsimd.partition_all_reduce(
                        zbc[:], zz[:], P, bass_isa.ReduceOp.add
                    )
                    recip = tpool.tile([P, NT_SIZE], F32, tag="recip", name="recip")
                    nc.vector.reciprocal(recip[:], zbc[:])
                    for cc in range(CCN):
                        outsb = tpool.tile([P, NT_SIZE], F32, tag="outsb", name="outsb")
                        nc.vector.tensor_mul(outsb[:], u_ps[nt][cc][:], recip[:])
                        nc.sync.dma_start(out[cc * P : (cc + 1) * P, ns], outsb[:])

                supers = [(nt, ms) for nt in range(NTN) for ms in range(MCN // 2)]
                S = len(supers)
                expt_q: dict = {}
                pending_tail = []
                dots_q = [emit_qk_super(*supers[0])]
                # iterate i: exp_i, QK_{i+1}, AV_{i-2}; AV lag 2 keeps only 2
                # dots supers in flight while giving exp a full super of slack
                for i in range(S + 2):
                    if i < S:
                        d2 = dots_q.pop(0)
                        if "exp" in ablate:
                            expt_q[i] = const_expt
                        else:
                            expt2 = wpool.tile([P, NT2], BF16, tag="expt2", name="expt2")
                            src_ap = cexpsrc if "expdep" in ablate else d2
                            nc.scalar.activation(
                                expt2[:], src_ap[:], EXP, bias=zero[:], scale=SCALE
                            )
                            expt_q[i] = expt2
                    if i + 1 < S:
                        dots_q.append(emit_qk_super(*supers[i + 1]))
                    j = i - 2
                    if j >= 0:
                        emit_av(j)
                        expt_q.pop(j)
                        nt, ms = supers[j]
                        if pending_tail and pending_tail[0][0] <= j:
                            emit_tail(pending_tail.pop(0)[1])
                        if ms == MCN // 2 - 1 and "tail" not in ablate and "z" not in ablate:
                            pending_tail.append((j + wtail_defer, nt))
                for due, nt in pending_tail:
                    emit_tail(nt)

            if wide and "exp" in ablate:
                # const_expt is [P, NT_SIZE]; wide path needs [P, 2*NT_SIZE]
                const_expt = cpool.tile([P, 2 * NT_SIZE], BF16, tag="cexpt2", name="cexpt2")
                nc.vector.memset(const_expt[:], 1.0)

            if loop_reps > 0:
                u = unroll
                while loop_reps % u:
                    u -= 1
                with tc.For_i(
                    0, loop_reps // u, 1,
                    hint_engines=(mybir.EngineType.PE,),
                    staggered_reset=True,
                ):
                    for _ in range(u):
                        (emit_body_wide if wide else emit_body)()
            else:
                for _ in range(reps):
                    (emit_body_wide if wide else emit_body)()

    nc.compile()
    return nc


def _prep_inputs(queries, keys, values, mask_eye, mask_mouth, q_pos, k_pos,
                 bias_eye, bias_mouth):
    """Host-side shard prep: positional adds, V transpose, the per-batch
    scalar bias folded into V as a constant row add, bf16 conversion."""
    q = queries.reshape(B, C, N) + q_pos[0].T[None]
    k = keys.reshape(B, C, N) + k_pos[0].T[None]
    vT = values.reshape(B, C, N).transpose(0, 2, 1)  # [B,N,C]

    def msum(mask):
        # nearest resize 128->64 picks every other row/col
        m = mask[:, :, ::2, ::2].reshape(B, -1)
        return (m * m).sum(axis=1, dtype=np.float64)

    softplus = lambda x: np.logaddexp(0.0, x)
    c_b = softplus(float(bias_eye[0]) * msum(mask_eye)) + softplus(
        float(bias_mouth[0]) * msum(mask_mouth)
    )  # [B]
    S = vT.sum(axis=1, dtype=np.float64)  # [B, C]
    cbs = c_b[:, None] * S  # [B, C]

    vaug = np.ascontiguousarray(
        (vT.astype(np.float64) + cbs[:, None, :]).astype(ml_dtypes.bfloat16)
    )  # [B,N,C]
    kb = k.astype(ml_dtypes.bfloat16)
    qb = q.astype(ml_dtypes.bfloat16)

    in_maps = []
    for core in range(NCORES):
        b, sh = divmod(core, SHARDS_PER_B)
        n0 = sh * NSH
        in_maps.append(
            {
                "keff": np.ascontiguousarray(kb[b]),
                "qeff": np.ascontiguousarray(qb[b][:, n0 : n0 + NSH]),
                "vaug": vaug[b],
            }
        )
    return in_maps


def kernel(**inputs) -> np.ndarray:
    inputs = {k: np.asarray(v, np.float32) for k, v in inputs.items()}
    in_maps = _prep_inputs(**inputs)

    if "nc" not in _CACHE:
        _CACHE["nc"] = _build_bass()
    res = run_bass_kernel_spmd(_CACHE["nc"], in_maps, list(range(NCORES)))

    full = np.empty((B, C, N), np.float32)
    for core in range(NCORES):
        b, sh = divmod(core, SHARDS_PER_B)
        n0 = sh * NSH
        full[b][:, n0 : n0 + NSH] = res.results[core]["out"]
    return full.reshape(B, C, H, W)

